# revision 5
# baseline (speedup 1.0000x reference)
"""Distributed brute-force KNN (retrieval) kernel for one TRN2 chip (8 NeuronCores).

Problem: queries [256,128] f32, candidates [500000,128] f32, identifiers [500000] i32,
k=100. Output: (values [256,100] f32 desc-sorted, ids [256,100] i32).

Strategy:
  - Shard candidates over N across the 8 cores (62500 each).
  - Per core: bf16 matmul (Q stationary, C^T shard streamed) -> PSUM scores
    [128q, 500c] tiles; DVE max/max_index extract top-8 (value,index) per
    500-candidate tile per query; results accumulate in SBUF, DMA'd out once.
  - Host: merge 8 cores' claimed top-8-per-tile candidates, rescore the
    contenders exactly in f32, and run a provable validation: any tile whose
    8th claimed value could still hide a top-k element is fully rescanned on
    host. Exactness never depends on device numerics.
"""
import numpy as np
import ml_dtypes

B = 256          # queries
N = 500000       # candidates
D = 128          # dim
NCORES = 8
NSH = N // NCORES          # 62500 per core
TILE = 500                 # candidates per psum tile
NTILES = NSH // TILE       # 125
CLAIM = NTILES * 8         # claimed entries per (core, query) = 1000

_CACHE = {}


def build(ntiles=NTILES, loops=1):
    """Build + compile the per-core Bass program. Returns the compiled Bacc."""
    import concourse.bass as bass
    import concourse.tile as tile
    from concourse import bacc, mybir

    bf16 = mybir.dt.bfloat16
    f32 = mybir.dt.float32
    u16 = mybir.dt.uint16
    nsh = ntiles * TILE

    nc = bacc.Bacc("TRN2", debug=False)
    qt = nc.dram_tensor("qt", [D, B], bf16, kind="ExternalInput").ap()
    ct = nc.dram_tensor("ct", [D, nsh], bf16, kind="ExternalInput").ap()
    v8 = nc.dram_tensor("v8", [B, ntiles * 8], f32, kind="ExternalOutput").ap()
    i8 = nc.dram_tensor("i8", [B, ntiles * 8], u16, kind="ExternalOutput").ap()

    CHUNK = 4  # ct tiles per DMA
    with tile.TileContext(nc) as tc:
        with (
            tc.tile_pool(name="qpool", bufs=1) as qpool,
            tc.tile_pool(name="cpool", bufs=3) as cpool,
            tc.tile_pool(name="psum", bufs=8, space="PSUM") as pp,
            tc.tile_pool(name="acc", bufs=1) as accp,
        ):
            qtile = qpool.tile([D, B], bf16)
            nc.sync.dma_start(qtile[:], qt[:])
            vacc = [
                accp.tile([128, ntiles * 8], f32, tag=f"vacc{h}", name=f"vacc{h}")
                for h in range(2)
            ]
            iacc = [
                accp.tile([128, ntiles * 8], u16, tag=f"iacc{h}", name=f"iacc{h}")
                for h in range(2)
            ]

            def body(_iv=None):
                for tt in range(0, ntiles, CHUNK):
                    nct = min(CHUNK, ntiles - tt)
                    ctile = cpool.tile([D, CHUNK * TILE], bf16, tag="ct")
                    nc.sync.dma_start(
                        ctile[:, 0 : nct * TILE],
                        ct[:, bass.ds(tt * TILE, nct * TILE)],
                    )
                    for j in range(nct):
                        t = tt + j
                        for h in range(2):
                            ps = pp.tile([128, TILE], f32)
                            nc.tensor.matmul(
                                ps[:],
                                lhsT=qtile[:, bass.ds(h * 128, 128)],
                                rhs=ctile[:, bass.ds(j * TILE, TILE)],
                                start=True,
                                stop=True,
                            )
                            vout = vacc[h][:, bass.ds(t * 8, 8)]
                            nc.vector.max(vout, ps[:])
                            nc.vector.max_index(iacc[h][:, bass.ds(t * 8, 8)], vout, ps[:])

            if loops == 1:
                body()
            else:
                with tc.For_i(0, loops, 1) as iv:
                    body(iv)

            for h in range(2):
                nc.sync.dma_start(v8[bass.ds(h * 128, 128), :], vacc[h][:])
                nc.sync.dma_start(i8[bass.ds(h * 128, 128), :], iacc[h][:])
    nc.compile()
    return nc


def _get_nc():
    if "nc" not in _CACHE:
        _CACHE["nc"] = build()
    return _CACHE["nc"]


def _device_claims(queries, candidates):
    """Run the 8-core SPMD kernel; return claimed (vals, gidx) [B, 8*CLAIM]."""
    from concourse.bass_utils import run_bass_kernel_spmd

    nc = _get_nc()
    qt = np.ascontiguousarray(queries.T).astype(ml_dtypes.bfloat16)
    cb = candidates.astype(ml_dtypes.bfloat16)
    in_maps = []
    for c in range(NCORES):
        ct = np.ascontiguousarray(cb[c * NSH : (c + 1) * NSH].T)
        in_maps.append({"qt": qt, "ct": ct})
    res = None
    for attempt in range(3):
        try:
            res = run_bass_kernel_spmd(nc, in_maps, core_ids=list(range(NCORES))).results
            break
        except Exception:
            if attempt == 2:
                raise
            import time as _time

            _time.sleep(2.0)
    assert res is not None
    v8 = np.stack([r["v8"] for r in res]).astype(np.float32)   # [8, B, CLAIM]
    i8 = np.stack([r["i8"] for r in res]).astype(np.int64)     # [8, B, CLAIM]
    offs = (np.arange(CLAIM) // 8) * TILE
    gidx = i8 + offs[None, None, :] + (np.arange(NCORES) * NSH)[:, None, None]
    vals = v8.transpose(1, 0, 2).reshape(B, NCORES * CLAIM)
    gidx = gidx.transpose(1, 0, 2).reshape(B, NCORES * CLAIM)
    return vals, gidx, v8


def kernel(queries, candidates, identifiers, k):
    queries = np.asarray(queries, dtype=np.float32)
    candidates = np.asarray(candidates, dtype=np.float32)
    identifiers = np.asarray(identifiers)
    kk = int(k)

    vals, gidx, v8 = _device_claims(queries, candidates)

    # --- host: exact rescore of contenders (f64 for stable ordering) -----
    q64 = queries.astype(np.float64)
    C = max(4 * kk, kk + 64)
    C = min(C, vals.shape[1] - 1)
    part = np.argpartition(-vals, C, axis=1)[:, :C]
    gsel = np.take_along_axis(gidx, part, 1)                   # [B, C]
    vsel = np.take_along_axis(vals, part, 1)
    se = np.einsum("qcd,qd->qc", candidates[gsel].astype(np.float64), q64)
    delta = np.abs(vsel - se).max(1)                           # device error bound/query
    sigma = np.linalg.norm(queries, axis=1)
    margin = 4.0 * delta + 1e-4 * sigma

    vk = -np.partition(-se, kk - 1, axis=1)[:, kk - 1]         # exact k-th value
    thr = vk - margin

    # pools per query as (value, gidx) arrays; start from rescored set
    pool_v = [se[q] for q in range(B)]
    pool_g = [gsel[q] for q in range(B)]

    # 1) any claimed entry above thr that wasn't rescored
    selmask = np.zeros(vals.shape, dtype=bool)
    np.put_along_axis(selmask, part, True, 1)
    need = (vals >= thr[:, None]) & ~selmask
    for q in np.nonzero(need.any(1))[0]:
        g = gidx[q, need[q]]
        sv = candidates[g].astype(np.float64) @ q64[q]
        pool_v[q] = np.concatenate([pool_v[q], sv])
        pool_g[q] = np.concatenate([pool_g[q], g])

    # 2) suspect tiles: 8th claimed value of a tile could hide unclaimed >= thr
    tmin = v8[:, :, 7::8]                                      # [8, B, NTILES]
    sus = tmin >= thr[None, :, None] - margin[None, :, None]   # extra slack
    for c, q, t in zip(*np.nonzero(sus)):
        base = c * NSH + t * TILE
        sv = candidates[base : base + TILE].astype(np.float64) @ q64[q]
        g = np.arange(base, base + TILE, dtype=np.int64)
        pool_v[q] = np.concatenate([pool_v[q], sv])
        pool_g[q] = np.concatenate([pool_g[q], g])

    # --- final exact top-k per query (dedupe, desc value, index tiebreak) --
    out_v = np.empty((B, kk), np.float32)
    out_g = np.empty((B, kk), np.int64)
    for q in range(B):
        g, first = np.unique(pool_g[q], return_index=True)
        v32 = pool_v[q][first].astype(np.float32)
        assert v32.size >= kk
        order = np.lexsort((g, -v32))[:kk]
        out_v[q] = v32[order]
        out_g[q] = g[order]

    top_ids = identifiers[out_g]
    return out_v, top_ids


# revision 8
# speedup vs baseline: 2.0626x; 2.0626x over previous
"""Distributed brute-force KNN (retrieval) kernel for one TRN2 chip (8 NeuronCores).

Problem: queries [256,128] f32, candidates [500000,128] f32, identifiers [500000] i32,
k=100. Output: (values [256,100] f32 desc-sorted, ids [256,100] i32).

Strategy:
  - Shard candidates over N across the 8 cores (62500 each).
  - Per core: bf16 matmul (Q stationary, C^T shard streamed) -> PSUM score
    tiles [128q, 500c]. ScalarE copies each tile to SBUF f32; VectorE folds
    it 500->250->125 with pairwise max (each folded slot covers a group of 4
    candidates), then max/max_index extract the top-8 (value, slot) per
    125-slot window per query. Claims accumulate in SBUF, one DMA out.
  - Host: expand each claimed slot to its 4 candidates, rescore contenders
    exactly in f64, and validate: any window whose 8th claimed value could
    still hide a top-k element is fully rescanned on host. Exactness never
    depends on device numerics.
"""
import numpy as np
import ml_dtypes

B = 256          # queries
N = 500000       # candidates
D = 128          # dim
NCORES = 8
NSH = N // NCORES          # 62500 per core
TILE = 500                 # candidates per psum tile
NTILES = NSH // TILE       # 125
FOLD = 4                   # candidates per claimed slot (two pairwise folds)
SLOTS = TILE // FOLD       # 125 slots per tile window
CLAIM = NTILES * 8         # claimed entries per (core, query) = 1000

_CACHE = {}


def build(ntiles=NTILES, loops=1, variant="fold"):
    """Build + compile the per-core Bass program. Returns the compiled Bacc."""
    import concourse.bass as bass
    import concourse.tile as tile
    from concourse import bacc, mybir

    bf16 = mybir.dt.bfloat16
    f32 = mybir.dt.float32
    u16 = mybir.dt.uint16
    Copy = mybir.ActivationFunctionType.Copy
    nsh = ntiles * TILE

    nc = bacc.Bacc("TRN2", debug=False)
    qt = nc.dram_tensor("qt", [D, B], bf16, kind="ExternalInput").ap()
    ct = nc.dram_tensor("ct", [D, nsh], bf16, kind="ExternalInput").ap()
    v8 = nc.dram_tensor("v8", [B, ntiles * 8], f32, kind="ExternalOutput").ap()
    i8 = nc.dram_tensor("i8", [B, ntiles * 8], u16, kind="ExternalOutput").ap()

    CHUNK = 4  # ct tiles per DMA
    with tile.TileContext(nc) as tc:
        with (
            tc.tile_pool(name="qpool", bufs=1) as qpool,
            tc.tile_pool(name="cpool", bufs=3) as cpool,
            tc.tile_pool(name="psum", bufs=8, space="PSUM") as pp,
            tc.tile_pool(name="fold", bufs=4) as fpool,
            tc.tile_pool(name="acc", bufs=1) as accp,
        ):
            qtile = qpool.tile([D, B], bf16)
            nc.sync.dma_start(qtile[:], qt[:])
            vacc = [
                accp.tile([128, ntiles * 8], f32, tag=f"vacc{h}", name=f"vacc{h}")
                for h in range(2)
            ]
            iacc = [
                accp.tile([128, ntiles * 8], u16, tag=f"iacc{h}", name=f"iacc{h}")
                for h in range(2)
            ]

            def body(_iv=None):
                for tt in range(0, ntiles, CHUNK):
                    nct = min(CHUNK, ntiles - tt)
                    ctile = cpool.tile([D, CHUNK * TILE], bf16, tag="ct", name="ctile")
                    nc.sync.dma_start(
                        ctile[:, 0 : nct * TILE],
                        ct[:, bass.ds(tt * TILE, nct * TILE)],
                    )
                    for j in range(nct):
                        t = tt + j
                        for h in range(2):
                            ps = pp.tile([128, TILE], f32, name="ps")
                            nc.tensor.matmul(
                                ps[:],
                                lhsT=qtile[:, bass.ds(h * 128, 128)],
                                rhs=ctile[:, bass.ds(j * TILE, TILE)],
                                start=True,
                                stop=True,
                            )
                            vout = vacc[h][:, bass.ds(t * 8, 8)]
                            iout = iacc[h][:, bass.ds(t * 8, 8)]
                            if variant == "base":
                                nc.vector.max(vout, ps[:])
                                nc.vector.max_index(iout, vout, ps[:])
                            elif variant == "fold":
                                f0 = fpool.tile([128, TILE], f32, tag="f0", name="f0")
                                nc.scalar.activation(f0[:], ps[:], Copy)
                                f1 = fpool.tile([128, TILE // 2], f32, tag="f1", name="f1")
                                nc.vector.tensor_max(
                                    f1[:], f0[:, bass.ds(0, 250)], f0[:, bass.ds(250, 250)]
                                )
                                f2 = fpool.tile([128, SLOTS], f32, tag="f2", name="f2")
                                nc.vector.tensor_max(
                                    f2[:], f1[:, bass.ds(0, 125)], f1[:, bass.ds(125, 125)]
                                )
                                nc.vector.max(vout, f2[:])
                                nc.vector.max_index(iout, vout, f2[:])
                            elif variant == "onepass":
                                nc.vector.max(vout, ps[:])
                            elif variant == "secopy":
                                f0 = fpool.tile([128, TILE], f32, tag="f0", name="f0")
                                nc.scalar.activation(f0[:], ps[:], Copy)
                                nc.vector.max(vout, f0[:])
                            else:
                                raise ValueError(variant)

            if loops == 1:
                body()
            else:
                with tc.For_i(0, loops, 1) as iv:
                    body(iv)

            for h in range(2):
                nc.sync.dma_start(v8[bass.ds(h * 128, 128), :], vacc[h][:])
                nc.sync.dma_start(i8[bass.ds(h * 128, 128), :], iacc[h][:])
    nc.compile()
    return nc


def _get_nc():
    if "nc" not in _CACHE:
        _CACHE["nc"] = build()
    return _CACHE["nc"]


def _device_claims(queries, candidates):
    """Run the 8-core SPMD kernel; return claimed (vals, slot gidx base) arrays."""
    from concourse.bass_utils import run_bass_kernel_spmd

    nc = _get_nc()
    qt = np.ascontiguousarray(queries.T).astype(ml_dtypes.bfloat16)
    cb = candidates.astype(ml_dtypes.bfloat16)
    in_maps = []
    for c in range(NCORES):
        ct = np.ascontiguousarray(cb[c * NSH : (c + 1) * NSH].T)
        in_maps.append({"qt": qt, "ct": ct})
    res = None
    for attempt in range(3):
        try:
            res = run_bass_kernel_spmd(nc, in_maps, core_ids=list(range(NCORES))).results
            break
        except Exception:
            if attempt == 2:
                raise
            import time as _time

            _time.sleep(2.0)
    assert res is not None
    v8 = np.stack([r["v8"] for r in res]).astype(np.float32)   # [8, B, CLAIM]
    i8 = np.stack([r["i8"] for r in res]).astype(np.int64)     # [8, B, CLAIM] slot in [0,SLOTS)
    # base candidate index of the claimed slot (first member of its group):
    # global = core*NSH + tile*TILE + slot (+ m*SLOTS for m in 0..FOLD-1)
    offs = (np.arange(CLAIM) // 8) * TILE
    gbase = i8 + offs[None, None, :] + (np.arange(NCORES) * NSH)[:, None, None]
    vals = v8.transpose(1, 0, 2).reshape(B, NCORES * CLAIM)
    gbase = gbase.transpose(1, 0, 2).reshape(B, NCORES * CLAIM)
    return vals, gbase, v8


def _expand(gb):
    """Expand claimed slot base indices [*, M] -> candidate indices [*, M*FOLD]."""
    return (gb[..., None] + (np.arange(FOLD) * SLOTS)[None, :]).reshape(*gb.shape[:-1], -1)


def kernel(queries, candidates, identifiers, k):
    queries = np.asarray(queries, dtype=np.float32)
    candidates = np.asarray(candidates, dtype=np.float32)
    identifiers = np.asarray(identifiers)
    kk = int(k)

    vals, gbase, v8 = _device_claims(queries, candidates)

    # --- host: exact rescore of contenders (f64 for stable ordering) -----
    q64 = queries.astype(np.float64)
    C = max(4 * kk, kk + 64)
    C = min(C, vals.shape[1] - 1)
    part = np.argpartition(-vals, C, axis=1)[:, :C]
    bsel = np.take_along_axis(gbase, part, 1)                  # [B, C] slot bases
    vsel = np.take_along_axis(vals, part, 1)
    gsel = _expand(bsel)                                       # [B, C*FOLD]
    se = np.einsum("qcd,qd->qc", candidates[gsel].astype(np.float64), q64)
    # claimed value ~ max over its group of FOLD exact scores
    se_g = se.reshape(B, C, FOLD)
    delta = np.abs(vsel - se_g.max(2)).max(1)                  # device error bound/query
    sigma = np.linalg.norm(queries, axis=1)
    margin = 4.0 * delta + 1e-4 * sigma

    vk = -np.partition(-se, kk - 1, axis=1)[:, kk - 1]         # exact k-th value
    thr = vk - margin

    pool_v = [se[q] for q in range(B)]
    pool_g = [gsel[q] for q in range(B)]

    # 1) any claimed entry above thr that wasn't rescored
    selmask = np.zeros(vals.shape, dtype=bool)
    np.put_along_axis(selmask, part, True, 1)
    need = (vals >= thr[:, None]) & ~selmask
    for q in np.nonzero(need.any(1))[0]:
        g = _expand(gbase[q, need[q]])
        sv = candidates[g].astype(np.float64) @ q64[q]
        pool_v[q] = np.concatenate([pool_v[q], sv])
        pool_g[q] = np.concatenate([pool_g[q], g])

    # 2) suspect windows: 8th claimed value of a tile could hide unclaimed >= thr,
    #    or duplicate claimed slots within a tile (value ties collapsing groups)
    tmin = v8[:, :, 7::8]                                      # [8, B, NTILES]
    sus = tmin >= (thr - margin)[None, :, None]
    # duplicate claimed slots in a window (f32 value tie collapsing groups):
    # rescan those windows too, if their value range can reach thr
    iw = np.sort(gbase.reshape(B, NCORES, NTILES, 8), axis=3)
    hasdup = (np.diff(iw, axis=3) == 0).any(3)                 # [B, 8, NTILES]
    vmax_w = v8[:, :, 0::8]                                    # top claimed per window
    dup_sus = hasdup.transpose(1, 0, 2) & (vmax_w >= (thr - margin)[None, :, None])
    sus = sus | dup_sus
    for c, q, t in zip(*np.nonzero(sus)):
        base = c * NSH + t * TILE
        sv = candidates[base : base + TILE].astype(np.float64) @ q64[q]
        g = np.arange(base, base + TILE, dtype=np.int64)
        pool_v[q] = np.concatenate([pool_v[q], sv])
        pool_g[q] = np.concatenate([pool_g[q], g])

    # --- final exact top-k per query (dedupe, desc value, index tiebreak) --
    out_v = np.empty((B, kk), np.float32)
    out_g = np.empty((B, kk), np.int64)
    for q in range(B):
        g, first = np.unique(pool_g[q], return_index=True)
        v32 = pool_v[q][first].astype(np.float32)
        assert v32.size >= kk
        order = np.lexsort((g, -v32))[:kk]
        out_v[q] = v32[order]
        out_g[q] = g[order]

    top_ids = identifiers[out_g]
    return out_v, top_ids
